# revision 60
# baseline (speedup 1.0000x reference)
"""Trainium2 Bass kernel for nn_MultiHeadAttention_8400956031164.

Full attention block: QKV proj + per-head RMSNorm + RoPE + sliding-window
causal attention (WIN=1024) + output proj.

Sharding: tensor-parallel over heads across 8 cores (2 heads/core), both
batches looped per core. Host sums the 8 partial Wo outputs.

Device-side layout strategy (per core):
  - X^T [D, S] streamed per 512-token group; Q,K produced TRANSPOSED
    [hd=128, s] per head directly from projection (lhsT = W slice).
  - All matmul operands in bf16 (PSUM accumulation f32): measured end-to-end
    rounding impact ~3.8e-3 rel err vs the 2e-2 gate; bf16 runs 1 cycle/row
    at any output width (f32r degrades 4x under 256) and halves DMA + SBUF +
    LDWEIGHTS time.
  - RMSNorm in transposed layout: sum(q^2) over hd via all-ones matmul
    (broadcast across partitions in PSUM), 1/sqrt via exp(-0.5*ln) on ACT.
  - RoPE in transposed layout: rotate_half runs on GpSimd (its SW cores
    may read a shifted partition base), with the sign AND the 64-row
    partition roll folded into the host sin tables; keeps the PE free.
  - Scores computed transposed S^T[k, q] (k on partitions) per 128x(<=512)
    block over the sliding window; exp on ACT (PSUM->SBUF, bf16 out);
    causal/window triangle masks applied as 0/1-table multiplies on DVE
    (GpSimd's in-order queue is busy with the rotates).
  - Softmax denominator via all-ones matmul accumulation; 1/L via the
    single-pass DVE reciprocal_approx_fast; PV accumulates V^T @ P^T =
    out^T [hd, q] in PSUM with variable-N has_written semantics.
  - Wo: lhsT = normalized out^T slices, accumulate 2 head-chunks, ACT/DVE
    copy PSUM->SBUF (bf16), DMA out bf16; host sums partials in f64.

Software pipeline (the key to keeping PE ~90% busy): per 512-token step N
the emission order is [input-DMA prefetch for N+2] + [proj chains of N,
with the attention blocks of step N-1 interleaved between chains] + [Wo
of step N-2]. The ACT exp stream of step N-1 then overlaps the PE
projection matmuls of step N instead of serializing behind them, and the
PE never waits on the softmax-normalization (DVE) tail. Deep tile pools
(x 6, out 6) decouple both DMA directions from compute: input transfers
start two steps before use, and Wo's PSUM->SBUF copies never wait on the
output-DMA drain. PSUM banks: 3 proj/Wo + 2 scores + 2 out + 1 denom.
"""

import functools

import numpy as np
import ml_dtypes
from contextlib import ExitStack

import concourse.bass as bass
import concourse.tile as tile
import concourse.mybir as mybir
import concourse.hw_specs as _hw_specs
from concourse import bacc, bass_utils


def _patch_act_tables():
    """Steer every activation in this kernel (Square/Ln/Exp/Copy) to the one
    ACT table set that really contains them all (natural_log_exp_and_others),
    so the greedy first-containing-set chooser never thrashes table loads.
    Only *removes* candidate sets; chosen ids still match act_info.json."""
    if getattr(_patch_act_tables, "done", False):
        return
    orig = _hw_specs.get_activation_tables
    AFT = mybir.ActivationFunctionType
    drop = {AFT.Exp, AFT.Ln, AFT.Square, AFT.Copy, AFT.Identity}

    @functools.cache
    def patched(module_arch):
        t = {k: set(v) for k, v in orig(module_arch).items()}
        for name, funcs in t.items():
            if name != "natural_log_exp_and_others":
                funcs -= drop
        return t

    _hw_specs.get_activation_tables = patched
    bacc.get_activation_tables = patched
    _patch_act_tables.done = True


_patch_act_tables()

B, S, D, H, HD, WIN = 2, 2048, 2048, 16, 128, 1024
EPS = 1e-6
SCALE = HD ** -0.5
NCORES = 8
HLOC = H // NCORES          # heads per core = 2
NL = HLOC * HD              # local head dims = 256
SG = 512                    # token group size
G = S // SG                 # groups per batch = 4
NDK = D // 128              # contraction chunks = 16

F32 = mybir.dt.float32
BF16 = mybir.dt.bfloat16
AF = mybir.ActivationFunctionType

_CACHE = {}


def _build():
    nc = bacc.Bacc(trn_type="TRN2", target_bir_lowering=False, debug=False)

    def din(name, shape, dt):
        return nc.dram_tensor(name, shape, dt, kind="ExternalInput").ap()

    # All inputs are host-pre-tiled to be partition-major contiguous so every
    # DMA is ~128 descriptors of large contiguous runs.
    xt = din("xt", [B * G, 128, NDK * SG], BF16)      # per (b,g) [128, 16*512]
    wq = din("wq", [128, NDK * NL], BF16)
    wk = din("wk", [128, NDK * NL], BF16)
    wv = din("wv", [128, NDK * NL], BF16)
    wo = din("wo", [128, HLOC * D], BF16)
    cs = din("cs", [B * G, 128, 4 * SG], BF16)        # packed cq|sq|ck|sk
    ones_d = din("ones_d", [128, 128], BF16)
    tri_d = din("tri_d", [128, 256], BF16)   # causal-keep | window-keep 0/1
    opart = nc.dram_tensor("opart", [B * S, D], BF16, kind="ExternalOutput").ap()

    steps = [(b, g) for b in range(B) for g in range(G)]
    NSTEP = len(steps)

    with tile.TileContext(nc) as tc, ExitStack() as ctx:
        const = ctx.enter_context(tc.tile_pool(name="const", bufs=1))
        wpool = ctx.enter_context(tc.tile_pool(name="w", bufs=1))
        xpool = ctx.enter_context(tc.tile_pool(name="x", bufs=6))
        cspool = ctx.enter_context(tc.tile_pool(name="cs", bufs=3))
        qpool = ctx.enter_context(tc.tile_pool(name="qr", bufs=5))
        kpool = ctx.enter_context(tc.tile_pool(name="kr", bufs=10))
        vpool = ctx.enter_context(tc.tile_pool(name="v", bufs=18))
        rpool = ctx.enter_context(tc.tile_pool(name="rms", bufs=2))
        ppool = ctx.enter_context(tc.tile_pool(name="p", bufs=5))
        lpool = ctx.enter_context(tc.tile_pool(name="lin", bufs=2))
        opool = ctx.enter_context(tc.tile_pool(name="osb", bufs=6))
        outp = ctx.enter_context(tc.tile_pool(name="out", bufs=6))
        psA = ctx.enter_context(tc.tile_pool(name="psA", bufs=2, space="PSUM"))
        psS = ctx.enter_context(tc.tile_pool(name="psS", bufs=3, space="PSUM"))
        psO = ctx.enter_context(tc.tile_pool(name="psO", bufs=2, space="PSUM"))
        psL = ctx.enter_context(tc.tile_pool(name="psL", bufs=1, space="PSUM"))

        inputs = {}    # idx -> (xh0, xh1, cst)
        qrs = {}       # idx -> {h: [128,SG] bf16}
        KrT = {}       # (b, h, g) -> [128,SG] bf16
        Vt = {}        # (b, st_abs) -> [128,NL] bf16
        osbs_all = {}  # idx -> {h: [128,SG] bf16}

        def issue_dmas(idx, spread=False):
            """Input DMAs for one step. `spread` (startup only) issues the
            chunks from different engines for parallel DMA queues."""
            b, g = steps[idx]
            bg = b * G + g
            src = xt[bg].rearrange("p (a s) -> p a s", s=SG)
            xh0 = xpool.tile([128, 8, SG], BF16, tag="xt", name="xh0")
            xh1 = xpool.tile([128, 8, SG], BF16, tag="xt", name="xh1")
            if spread:
                nc.sync.dma_start(xh0[:, 0:3, :], src[:, 0:3, :])
                nc.gpsimd.dma_start(xh0[:, 3:5, :], src[:, 3:5, :])
                nc.scalar.dma_start(xh0[:, 5:8, :], src[:, 5:8, :])
                nc.gpsimd.dma_start(xh1[:], src[:, 8:16, :])
            else:
                nc.sync.dma_start(xh0[:], src[:, 0:8, :])
                nc.sync.dma_start(xh1[:], src[:, 8:16, :])
            cst = cspool.tile([128, 4, SG], BF16, tag="cs")
            nc.sync.dma_start(
                cst[:], cs[bg].rearrange("p (f s) -> p f s", s=SG))
            inputs[idx] = (xh0, xh1, cst)

        def attn_thunks(idx):
            """Attention micro-ops (PE-centric) for step idx, as a list of
            thunks to interleave between the next step's projection chains."""
            b, g = steps[idx]
            qr_tiles = qrs[idx]
            thunks = []
            osbs = {}
            osbs_all[idx] = osbs

            def make_head(h):
                qr_t = qr_tiles[h]
                kts = list(range(max(0, 4 * g - 8), 4 * g + 4))
                nk = len(kts)
                state = {}
                pend = []
                LAG = 3

                def start():
                    state["oacc"] = psO.tile([128, SG], F32, tag="o",
                                             name="oacc")
                    state["lacc"] = psL.tile([128, SG], F32, tag="l",
                                             name="lacc")

                def emit_pv(item, first, last):
                    kt, qoff, n, p = item
                    nc.tensor.matmul(
                        state["oacc"][:, qoff:qoff + n],
                        Vt[(b, kt)][:, h * HD:(h + 1) * HD], p[:],
                        start=first, stop=last)
                    nc.tensor.matmul(
                        state["lacc"][:, qoff:qoff + n], ones_t[:], p[:],
                        start=first, stop=last)

                def block(i):
                    kt = kts[i]
                    qt_lo = max(4 * g, kt)
                    qt_hi = min(4 * g + 3, kt + 8)
                    qoff = 128 * (qt_lo - 4 * g)
                    n = 128 * (qt_hi - qt_lo + 1)
                    sc = psS.tile([128, n], F32, tag="score")
                    kr_t = KrT[(b, h, kt // 4)]
                    c = (kt % 4) * 128
                    nc.tensor.matmul(sc[:], kr_t[:, c:c + 128],
                                     qr_t[:, qoff:qoff + n],
                                     start=True, stop=True)
                    p = ppool.tile([128, n], BF16, tag="p")
                    nc.scalar.activation(p[:], sc[:], AF.Exp)
                    if kt >= 4 * g:
                        # causal triangle: keep kk <= qq (0/1 mul on DVE —
                        # GpSimd is busy with the rotate muls)
                        nc.vector.tensor_mul(p[:, 0:128], p[:, 0:128],
                                             tri_t[:, 0:128])
                    if kt + 8 <= 4 * g + 3:
                        # window edge: keep kk >= qq
                        nc.vector.tensor_mul(p[:, n - 128:n],
                                             p[:, n - 128:n],
                                             tri_t[:, 128:256])
                    pend.append((kt, qoff, n, p))
                    if i >= LAG:
                        emit_pv(pend[i - LAG], first=(i - LAG == 0), last=False)

                def tail():
                    for j in range(max(0, nk - LAG), nk):
                        emit_pv(pend[j], first=(j == 0), last=(j == nk - 1))
                    linv = lpool.tile([128, SG], F32, tag="lin")
                    nc.vector.reciprocal_approx_fast(linv[:], state["lacc"][:])
                    osb = opool.tile([128, SG], BF16, tag="osb")
                    nc.vector.tensor_mul(osb[:], state["oacc"][:], linv[:])
                    osbs[h] = osb

                thunks.append(start)
                for i in range(nk):
                    thunks.append(lambda i=i: block(i))
                thunks.append(tail)

            for h in range(HLOC):
                make_head(h)
            return thunks

        def wo_thunks(idx):
            """Wo micro-ops for step idx as thunks: one per (st, dg) psum
            pair, plus the output DMA after each st row completes."""
            b, g = steps[idx]
            s0 = b * S + g * SG
            osbs_ = osbs_all.pop(idx)
            thunks = []
            state = {}

            def pair(st, dg):
                if dg == 0:
                    state["ot"] = outp.tile([128, D], BF16, tag="out",
                                            name="ot")
                pso = psA.tile([128, 512], F32, tag="a")
                nc.tensor.matmul(pso[:],
                                 osbs_[0][:, st * 128:(st + 1) * 128],
                                 wo_t[:, 0, dg * 512:(dg + 1) * 512],
                                 start=True, stop=False)
                nc.tensor.matmul(pso[:],
                                 osbs_[1][:, st * 128:(st + 1) * 128],
                                 wo_t[:, 1, dg * 512:(dg + 1) * 512],
                                 start=False, stop=True)
                dst = state["ot"][:, dg * 512:(dg + 1) * 512]
                if dg % 2 == 0:
                    nc.scalar.copy(dst, pso[:])
                else:
                    nc.vector.tensor_copy(dst, pso[:])
                if dg == 3:
                    row = s0 + st * 128
                    nc.sync.dma_start(opart[row:row + 128, :],
                                      state["ot"][:])

            for st in range(4):
                for dg in range(4):
                    thunks.append(lambda st=st, dg=dg: pair(st, dg))
            return thunks

        # DMA issue order = first-use order, split into small tiles so the
        # PE's first projection matmuls start as soon as the first x / wq
        # chunks land (deps are tile-granular). Startup chunks ride
        # different engines' issue queues for parallel DMA; wo_t is not
        # needed until step 2.
        wqsrc = wq.rearrange("p (a n) -> p a n", n=NL)
        wq_t = wpool.tile([128, NDK, NL], BF16, tag="wq")
        nc.sync.dma_start(wq_t[:, 0:6, :], wqsrc[:, 0:6, :])
        nc.gpsimd.dma_start(wq_t[:, 6:11, :], wqsrc[:, 6:11, :])
        nc.scalar.dma_start(wq_t[:, 11:16, :], wqsrc[:, 11:16, :])
        issue_dmas(0, spread=True)
        wk_t = wpool.tile([128, NDK, NL], BF16, tag="wk")
        nc.sync.dma_start(wk_t[:], wk.rearrange("p (a n) -> p a n", n=NL))
        ones_t = const.tile([128, 128], BF16, tag="ones")
        nc.sync.dma_start(ones_t[:], ones_d)
        eps_t = const.tile([128, 1], F32, tag="eps")
        nc.vector.memset(eps_t[:], EPS)
        wv_t = wpool.tile([128, NDK, NL], BF16, tag="wv")
        nc.sync.dma_start(wv_t[:], wv.rearrange("p (a n) -> p a n", n=NL))
        tri_t = const.tile([128, 256], BF16, tag="tri")
        nc.sync.dma_start(tri_t[:], tri_d)
        issue_dmas(1)
        wo_t = wpool.tile([128, HLOC, D], BF16, tag="wo")
        nc.sync.dma_start(wo_t[:], wo.rearrange("p (c d) -> p c d", d=D))
        issue_dmas(2)
        for idx in range(NSTEP):
            b, g = steps[idx]
            if 1 <= idx and idx + 2 < NSTEP:
                issue_dmas(idx + 2)
            xh0, xh1, cst = inputs.pop(idx)
            cqt = cst[:, 0, :]
            sqt = cst[:, 1, :]
            ckt = cst[:, 2, :]
            skt = cst[:, 3, :]

            def xs(dk):
                t = xh0 if dk < 8 else xh1
                return t[:, dk % 8, :]

            # attention thunks of the previous step, spread over this step's
            # projection chains
            th = attn_thunks(idx - 1) if idx > 0 else []
            tpos = 0

            def run_thunks(target):
                nonlocal tpos
                while tpos < min(target, len(th)):
                    th[tpos]()
                    tpos += 1

            # ---- Q/K transposed projections + RMSNorm + RoPE ----
            # PE pipelining: after each projection chain, emit the previous
            # chain's sum-of-squares matmul and the chain before that's
            # rotate matmul, so PE never waits on ACT/DVE.
            qr_tiles = {}
            qrs[idx] = qr_tiles
            states = []

            def emit_ss(stt):
                ssps = psS.tile([128, SG], F32, tag="score")
                nc.tensor.matmul(ssps[:], ones_t[:], stt["qsq"][:],
                                 start=True, stop=True)
                # 1/sqrt(v) = exp(-0.5*ln(v)) keeps every ACT func in the
                # natural_log_exp_and_others table set (no table thrash).
                rstd = rpool.tile([128, SG], F32, tag="rstd")
                nc.scalar.activation(rstd[:], ssps[:], AF.Ln,
                                     bias=eps_t[:, 0:1], scale=1.0 / HD)
                nc.scalar.activation(rstd[:], rstd[:], AF.Exp, scale=-0.5)
                qn = rpool.tile([128, SG], BF16, tag="qn")
                nc.vector.tensor_mul(qn[:], stt["ps"][:], rstd[:])
                t1 = rpool.tile([128, SG], BF16, tag="t1")
                cost = cqt if stt["t"] == "q" else ckt
                nc.vector.tensor_mul(t1[:], qn[:], cost[:])
                stt["qn"] = qn
                stt["t1"] = t1

            def emit_rot(stt):
                # rotate_half on GpSimd (SW cores may read a different
                # partition base than they write, as long as both INPUTS
                # share a base): dst[p] = qn[(p+64)%128] * sin_signed[p].
                # The sin tables are partition-rolled by 64 and sign-folded
                # on the host so both inputs align at the same base.
                sint = sqt if stt["t"] == "q" else skt
                dst = stt["dst"]
                qn = stt["qn"]
                nc.gpsimd.tensor_mul(dst[0:64, :], qn[64:128, :],
                                     sint[64:128, :])
                nc.gpsimd.tensor_mul(dst[64:128, :], qn[0:64, :],
                                     sint[0:64, :])
                nc.vector.tensor_add(dst[:], dst[:], stt["t1"][:])

            chains = [("q", 0), ("k", 0), ("q", 1), ("k", 1),
                      ("v", 0), ("v", 1), ("v", 2), ("v", 3)]
            for i, (t, h) in enumerate(chains):
                if t in ("q", "k"):
                    w_t = wq_t if t == "q" else wk_t
                    ps = psA.tile([128, SG], F32, tag="a")
                    for dk in range(NDK):
                        nc.tensor.matmul(
                            ps[:], w_t[:, dk, h * HD:(h + 1) * HD], xs(dk),
                            start=(dk == 0), stop=(dk == NDK - 1))
                    qsq = rpool.tile([128, SG], BF16, tag="qsq")
                    nc.scalar.activation(qsq[:], ps[:], AF.Square)
                    if t == "q":
                        dst = qpool.tile([128, SG], BF16, tag="qr")
                        qr_tiles[h] = dst
                    else:
                        dst = kpool.tile([128, SG], BF16, tag="kr")
                        KrT[(b, h, g)] = dst
                    states.append({"ps": ps, "qsq": qsq, "t": t, "dst": dst})
                else:
                    st = h
                    psv = psA.tile([128, NL], F32, tag="a")
                    for dk in range(NDK):
                        nc.tensor.matmul(
                            psv[:], xs(dk)[:, st * 128:(st + 1) * 128],
                            wv_t[:, dk, :],
                            start=(dk == 0), stop=(dk == NDK - 1))
                    vt = vpool.tile([128, NL], BF16, tag="v")
                    nc.vector.tensor_copy(vt[:], psv[:])
                    Vt[(b, 4 * g + st)] = vt
                if 0 <= i - 1 < 4:
                    emit_ss(states[i - 1])
                if 0 <= i - 2 < 4:
                    emit_rot(states[i - 2])
                run_thunks((len(th) * (i + 1)) // len(chains))

            run_thunks(len(th))

            # Wo of step idx-2 (its attention completed during step idx-1)
            if idx - 2 >= 0:
                for t in wo_thunks(idx - 2):
                    t()

        # drain: attention of the last step, then the last two Wo blocks
        for t in attn_thunks(NSTEP - 1) + wo_thunks(NSTEP - 2):
            t()
        for t in wo_thunks(NSTEP - 1):
            t()

    nc.compile()
    return nc


def _host_prep(hidden_states, cos, sin, Wq, Wk, Wv, Wo, q_scale, k_scale):
    f32 = np.float32
    bf16 = ml_dtypes.bfloat16
    hs = np.asarray(hidden_states, f32)
    cos = np.asarray(cos, f32)
    sin = np.asarray(sin, f32)
    qs = np.asarray(q_scale, f32)
    ks = np.asarray(k_scale, f32)

    def ptile(a2d, width):
        """[128*K, W] -> [128, K*W] partition-major contiguous pre-tiling."""
        k = a2d.shape[0] // 128
        return np.ascontiguousarray(
            a2d.reshape(k, 128, width).transpose(1, 0, 2).reshape(128, -1)
        ).astype(bf16)

    # xt: per (b,g) block of X^T, pre-tiled
    xt = np.stack([
        ptile(hs[b].T[:, g * SG:(g + 1) * SG], SG)
        for b in range(B) for g in range(G)
    ])   # [B*G, 128, 16*SG]

    qs_rot = np.roll(qs, -64)
    ks_rot = np.roll(ks, -64)
    # rotate_half's -1 on the first half is folded into the sin tables,
    # which are then partition-rolled by 64 so the GpSimd rotate muls read
    # both inputs (qn, sin) at the same partition base.
    sgn = np.ones((HD, 1), f32)
    sgn[:64] = -1.0
    cq_full = [(cos[b] * qs[None, :] * SCALE).T for b in range(B)]    # [HD,S]
    sq_full = [np.roll((sin[b] * qs_rot[None, :] * SCALE).T * sgn, 64, axis=0)
               for b in range(B)]
    ck_full = [(cos[b] * ks[None, :]).T for b in range(B)]
    sk_full = [np.roll((sin[b] * ks_rot[None, :]).T * sgn, 64, axis=0)
               for b in range(B)]
    cs_all = np.stack([
        np.concatenate([t[:, g * SG:(g + 1) * SG]
                        for t in (cq_full[b], sq_full[b],
                                  ck_full[b], sk_full[b])], axis=1)
        for b in range(B) for g in range(G)
    ]).astype(bf16)   # [B*G, 128, 4*SG]
    cs_all = np.ascontiguousarray(cs_all)

    ones = np.ones((128, 128), bf16)
    kk = np.arange(128)[:, None]
    qq = np.arange(128)[None, :]
    tri = np.concatenate([(qq >= kk).astype(f32),     # causal keep
                          (kk >= qq).astype(f32)],    # window-edge keep
                         axis=1).astype(bf16)
    shared = {"xt": xt, "cs": cs_all, "ones_d": ones, "tri_d": tri}
    Wq = np.asarray(Wq, f32)
    Wk = np.asarray(Wk, f32)
    Wv = np.asarray(Wv, f32)
    Wo = np.asarray(Wo, f32)
    in_maps = []
    for c in range(NCORES):
        m = dict(shared)
        m["wq"] = ptile(Wq[:, c * NL:(c + 1) * NL], NL)
        m["wk"] = ptile(Wk[:, c * NL:(c + 1) * NL], NL)
        m["wv"] = ptile(Wv[:, c * NL:(c + 1) * NL], NL)
        m["wo"] = ptile(Wo[c * NL:(c + 1) * NL, :], D)
        in_maps.append(m)
    return in_maps


def get_nc():
    if "nc" not in _CACHE:
        _CACHE["nc"] = _build()
    return _CACHE["nc"]


def kernel(hidden_states, cos, sin, Wq, Wk, Wv, Wo, q_scale, k_scale):
    nc = get_nc()
    in_maps = _host_prep(hidden_states, cos, sin, Wq, Wk, Wv, Wo,
                         q_scale, k_scale)
    res = bass_utils.run_bass_kernel_spmd(nc, in_maps,
                                          core_ids=list(range(NCORES)))
    acc = np.zeros((B * S, D), np.float64)
    for r in res.results:
        acc += r["opart"].astype(np.float64)
    return np.ascontiguousarray(
        acc.reshape(B, S, D).astype(np.float32))


# revision 62
# speedup vs baseline: 1.1135x; 1.1135x over previous
"""Trainium2 Bass kernel for nn_MultiHeadAttention_8400956031164.

Full attention block: QKV proj + per-head RMSNorm + RoPE + sliding-window
causal attention (WIN=1024) + output proj.

Sharding: tensor-parallel over heads across 8 cores (2 heads/core), both
batches looped per core. Host sums the 8 partial Wo outputs.

Device-side layout strategy (per core):
  - X^T [D, S] streamed per 512-token group; Q,K produced TRANSPOSED
    [hd=128, s] per head directly from projection (lhsT = W slice).
  - All matmul operands in bf16 (PSUM accumulation f32): measured end-to-end
    rounding impact ~3.8e-3 rel err vs the 2e-2 gate; bf16 runs 1 cycle/row
    at any output width (f32r degrades 4x under 256) and halves DMA + SBUF +
    LDWEIGHTS time.
  - RMSNorm in transposed layout: sum(q^2) over hd via all-ones matmul
    (broadcast across partitions in PSUM), 1/sqrt via exp(-0.5*ln) on ACT.
  - RoPE in transposed layout: rotate_half runs on GpSimd (its SW cores
    may read a shifted partition base), with the sign AND the 64-row
    partition roll folded into the host sin tables; keeps the PE free.
  - Scores computed transposed S^T[k, q] (k on partitions) per 128x(<=512)
    block over the sliding window; exp on ACT (PSUM->SBUF, bf16 out);
    causal/window triangle masks applied as 0/1-table multiplies on DVE
    (GpSimd's in-order queue is busy with the rotates).
  - Softmax denominator via all-ones matmul accumulation; 1/L via the
    single-pass DVE reciprocal_approx_fast; PV accumulates V^T @ P^T =
    out^T [hd, q] in PSUM with variable-N has_written semantics.
  - Wo: lhsT = normalized out^T slices, accumulate 2 head-chunks, ACT/DVE
    copy PSUM->SBUF (bf16), DMA out bf16; host sums partials in f64.

Software pipeline (the key to keeping PE ~90% busy): per 512-token step N
the emission order is [input-DMA prefetch for N+2] + [proj chains of N,
with the attention blocks of step N-1 interleaved between chains] + [Wo
of step N-2]. The ACT exp stream of step N-1 then overlaps the PE
projection matmuls of step N instead of serializing behind them, and the
PE never waits on the softmax-normalization (DVE) tail. Deep tile pools
(x 6, out 6) decouple both DMA directions from compute: input transfers
start two steps before use, and Wo's PSUM->SBUF copies never wait on the
output-DMA drain. PSUM banks: 3 proj/Wo + 2 scores + 2 out + 1 denom.
"""

import functools

import numpy as np
import ml_dtypes
from contextlib import ExitStack

import concourse.bass as bass
import concourse.tile as tile
import concourse.mybir as mybir
import concourse.hw_specs as _hw_specs
from concourse import bacc, bass_utils


def _patch_act_tables():
    """Steer every activation in this kernel (Square/Ln/Exp/Copy) to the one
    ACT table set that really contains them all (natural_log_exp_and_others),
    so the greedy first-containing-set chooser never thrashes table loads.
    Only *removes* candidate sets; chosen ids still match act_info.json."""
    if getattr(_patch_act_tables, "done", False):
        return
    orig = _hw_specs.get_activation_tables
    AFT = mybir.ActivationFunctionType
    drop = {AFT.Exp, AFT.Ln, AFT.Square, AFT.Copy, AFT.Identity}

    @functools.cache
    def patched(module_arch):
        t = {k: set(v) for k, v in orig(module_arch).items()}
        for name, funcs in t.items():
            if name != "natural_log_exp_and_others":
                funcs -= drop
        return t

    _hw_specs.get_activation_tables = patched
    bacc.get_activation_tables = patched
    _patch_act_tables.done = True


_patch_act_tables()

B, S, D, H, HD, WIN = 2, 2048, 2048, 16, 128, 1024
EPS = 1e-6
SCALE = HD ** -0.5
NCORES = 8
HLOC = H // NCORES          # heads per core = 2
NL = HLOC * HD              # local head dims = 256
SG = 512                    # token group size
G = S // SG                 # groups per batch = 4
NDK = D // 128              # contraction chunks = 16

F32 = mybir.dt.float32
BF16 = mybir.dt.bfloat16
AF = mybir.ActivationFunctionType

_CACHE = {}


def _build():
    nc = bacc.Bacc(trn_type="TRN2", target_bir_lowering=False, debug=False)

    def din(name, shape, dt):
        return nc.dram_tensor(name, shape, dt, kind="ExternalInput").ap()

    # All inputs are host-pre-tiled to be partition-major contiguous so every
    # DMA is ~128 descriptors of large contiguous runs.
    xt = din("xt", [B * G, 128, NDK * SG], BF16)      # per (b,g) [128, 16*512]
    wq = din("wq", [128, NDK * NL], BF16)
    wk = din("wk", [128, NDK * NL], BF16)
    wv = din("wv", [128, NDK * NL], BF16)
    wo = din("wo", [128, HLOC * D], BF16)
    cs = din("cs", [B * G, 128, 4 * SG], BF16)        # packed cq|sq|ck|sk
    ones_d = din("ones_d", [128, 128], BF16)
    tri_d = din("tri_d", [128, 256], BF16)   # causal-keep | window-keep 0/1
    opart = nc.dram_tensor("opart", [B * S, D], BF16, kind="ExternalOutput").ap()

    steps = [(b, g) for b in range(B) for g in range(G)]
    NSTEP = len(steps)

    with tile.TileContext(nc) as tc, ExitStack() as ctx:
        const = ctx.enter_context(tc.tile_pool(name="const", bufs=1))
        wpool = ctx.enter_context(tc.tile_pool(name="w", bufs=1))
        xpool = ctx.enter_context(tc.tile_pool(name="x", bufs=6))
        cspool = ctx.enter_context(tc.tile_pool(name="cs", bufs=3))
        qpool = ctx.enter_context(tc.tile_pool(name="qr", bufs=5))
        kpool = ctx.enter_context(tc.tile_pool(name="kr", bufs=10))
        vpool = ctx.enter_context(tc.tile_pool(name="v", bufs=18))
        rpool = ctx.enter_context(tc.tile_pool(name="rms", bufs=2))
        ppool = ctx.enter_context(tc.tile_pool(name="p", bufs=5))
        lpool = ctx.enter_context(tc.tile_pool(name="lin", bufs=2))
        opool = ctx.enter_context(tc.tile_pool(name="osb", bufs=6))
        outp = ctx.enter_context(tc.tile_pool(name="out", bufs=6))
        psA = ctx.enter_context(tc.tile_pool(name="psA", bufs=3, space="PSUM"))
        psS = ctx.enter_context(tc.tile_pool(name="psS", bufs=2, space="PSUM"))
        psO = ctx.enter_context(tc.tile_pool(name="psO", bufs=2, space="PSUM"))
        psL = ctx.enter_context(tc.tile_pool(name="psL", bufs=1, space="PSUM"))

        inputs = {}    # idx -> (xh0, xh1, cst)
        qrs = {}       # idx -> {h: [128,SG] bf16}
        KrT = {}       # (b, h, g) -> [128,SG] bf16
        Vt = {}        # (b, st_abs) -> [128,NL] bf16
        osbs_all = {}  # idx -> {h: [128,SG] bf16}

        def issue_dmas(idx, spread=False):
            """Input DMAs for one step. `spread` (startup only) issues the
            chunks from different engines for parallel DMA queues."""
            b, g = steps[idx]
            bg = b * G + g
            src = xt[bg].rearrange("p (a s) -> p a s", s=SG)
            xh0 = xpool.tile([128, 8, SG], BF16, tag="xt", name="xh0")
            xh1 = xpool.tile([128, 8, SG], BF16, tag="xt", name="xh1")
            if spread:
                nc.sync.dma_start(xh0[:, 0:3, :], src[:, 0:3, :])
                nc.gpsimd.dma_start(xh0[:, 3:5, :], src[:, 3:5, :])
                nc.scalar.dma_start(xh0[:, 5:8, :], src[:, 5:8, :])
                nc.gpsimd.dma_start(xh1[:], src[:, 8:16, :])
            else:
                nc.sync.dma_start(xh0[:], src[:, 0:8, :])
                nc.sync.dma_start(xh1[:], src[:, 8:16, :])
            cst = cspool.tile([128, 4, SG], BF16, tag="cs")
            nc.sync.dma_start(
                cst[:], cs[bg].rearrange("p (f s) -> p f s", s=SG))
            inputs[idx] = (xh0, xh1, cst)

        def attn_thunks(idx):
            """Attention micro-ops (PE-centric) for step idx, as a list of
            thunks to interleave between the next step's projection chains."""
            b, g = steps[idx]
            qr_tiles = qrs[idx]
            thunks = []
            osbs = {}
            osbs_all[idx] = osbs

            def make_head(h):
                qr_t = qr_tiles[h]
                kts = list(range(max(0, 4 * g - 8), 4 * g + 4))
                nk = len(kts)
                state = {}
                pend = []
                LAG = 3

                def start():
                    state["oacc"] = psO.tile([128, SG], F32, tag="o",
                                             name="oacc")
                    state["lacc"] = psL.tile([128, SG], F32, tag="l",
                                             name="lacc")

                def emit_pv(item, first, last):
                    kt, qoff, n, p = item
                    nc.tensor.matmul(
                        state["oacc"][:, qoff:qoff + n],
                        Vt[(b, kt)][:, h * HD:(h + 1) * HD], p[:],
                        start=first, stop=last)
                    nc.tensor.matmul(
                        state["lacc"][:, qoff:qoff + n], ones_t[:], p[:],
                        start=first, stop=last)

                def block(i):
                    kt = kts[i]
                    qt_lo = max(4 * g, kt)
                    qt_hi = min(4 * g + 3, kt + 8)
                    qoff = 128 * (qt_lo - 4 * g)
                    n = 128 * (qt_hi - qt_lo + 1)
                    sc = psS.tile([128, n], F32, tag="score")
                    kr_t = KrT[(b, h, kt // 4)]
                    c = (kt % 4) * 128
                    nc.tensor.matmul(sc[:], kr_t[:, c:c + 128],
                                     qr_t[:, qoff:qoff + n],
                                     start=True, stop=True)
                    p = ppool.tile([128, n], BF16, tag="p")
                    nc.scalar.activation(p[:], sc[:], AF.Exp)
                    if kt >= 4 * g:
                        # causal triangle: keep kk <= qq (0/1 mul on DVE —
                        # GpSimd is busy with the rotate muls)
                        nc.vector.tensor_mul(p[:, 0:128], p[:, 0:128],
                                             tri_t[:, 0:128])
                    if kt + 8 <= 4 * g + 3:
                        # window edge: keep kk >= qq
                        nc.vector.tensor_mul(p[:, n - 128:n],
                                             p[:, n - 128:n],
                                             tri_t[:, 128:256])
                    pend.append((kt, qoff, n, p))
                    if i >= LAG:
                        emit_pv(pend[i - LAG], first=(i - LAG == 0), last=False)

                def tail():
                    for j in range(max(0, nk - LAG), nk):
                        emit_pv(pend[j], first=(j == 0), last=(j == nk - 1))
                    linv = lpool.tile([128, SG], F32, tag="lin")
                    nc.vector.reciprocal_approx_fast(linv[:], state["lacc"][:])
                    osb = opool.tile([128, SG], BF16, tag="osb")
                    nc.vector.tensor_mul(osb[:], state["oacc"][:], linv[:])
                    osbs[h] = osb

                thunks.append(start)
                for i in range(nk):
                    thunks.append(lambda i=i: block(i))
                thunks.append(tail)

            for h in range(HLOC):
                make_head(h)
            return thunks

        def wo_thunks(idx):
            """Wo micro-ops for step idx as thunks: one per (st, dg) psum
            pair, plus the output DMA after each st row completes."""
            b, g = steps[idx]
            s0 = b * S + g * SG
            osbs_ = osbs_all.pop(idx)
            thunks = []
            state = {}

            def pair(st, dg):
                if dg == 0:
                    state["ot"] = outp.tile([128, D], BF16, tag="out",
                                            name="ot")
                pso = psA.tile([128, 512], F32, tag="a")
                nc.tensor.matmul(pso[:],
                                 osbs_[0][:, st * 128:(st + 1) * 128],
                                 wo_t[:, 0, dg * 512:(dg + 1) * 512],
                                 start=True, stop=False)
                nc.tensor.matmul(pso[:],
                                 osbs_[1][:, st * 128:(st + 1) * 128],
                                 wo_t[:, 1, dg * 512:(dg + 1) * 512],
                                 start=False, stop=True)
                dst = state["ot"][:, dg * 512:(dg + 1) * 512]
                if dg == 0:
                    # 1 of 4 on ACT, rest on DVE: ACT paces the exp stream
                    nc.scalar.copy(dst, pso[:])
                else:
                    nc.vector.tensor_copy(dst, pso[:])
                if dg == 3:
                    row = s0 + st * 128
                    nc.sync.dma_start(opart[row:row + 128, :],
                                      state["ot"][:])

            for st in range(4):
                for dg in range(4):
                    thunks.append(lambda st=st, dg=dg: pair(st, dg))
            return thunks

        # DMA issue order = first-use order, split into small tiles so the
        # PE's first projection matmuls start as soon as the first x / wq
        # chunks land (deps are tile-granular). Startup chunks ride
        # different engines' issue queues for parallel DMA; wo_t is not
        # needed until step 2.
        wqsrc = wq.rearrange("p (a n) -> p a n", n=NL)
        wq_t = wpool.tile([128, NDK, NL], BF16, tag="wq")
        nc.sync.dma_start(wq_t[:, 0:6, :], wqsrc[:, 0:6, :])
        nc.gpsimd.dma_start(wq_t[:, 6:11, :], wqsrc[:, 6:11, :])
        nc.scalar.dma_start(wq_t[:, 11:16, :], wqsrc[:, 11:16, :])
        issue_dmas(0, spread=True)
        wk_t = wpool.tile([128, NDK, NL], BF16, tag="wk")
        nc.sync.dma_start(wk_t[:], wk.rearrange("p (a n) -> p a n", n=NL))
        ones_t = const.tile([128, 128], BF16, tag="ones")
        nc.sync.dma_start(ones_t[:], ones_d)
        eps_t = const.tile([128, 1], F32, tag="eps")
        nc.vector.memset(eps_t[:], EPS)
        wv_t = wpool.tile([128, NDK, NL], BF16, tag="wv")
        nc.sync.dma_start(wv_t[:], wv.rearrange("p (a n) -> p a n", n=NL))
        tri_t = const.tile([128, 256], BF16, tag="tri")
        nc.sync.dma_start(tri_t[:], tri_d)
        issue_dmas(1)
        wo_t = wpool.tile([128, HLOC, D], BF16, tag="wo")
        nc.sync.dma_start(wo_t[:], wo.rearrange("p (c d) -> p c d", d=D))
        issue_dmas(2)
        for idx in range(NSTEP):
            b, g = steps[idx]
            if 1 <= idx and idx + 2 < NSTEP:
                issue_dmas(idx + 2)
            xh0, xh1, cst = inputs.pop(idx)
            cqt = cst[:, 0, :]
            sqt = cst[:, 1, :]
            ckt = cst[:, 2, :]
            skt = cst[:, 3, :]

            def xs(dk):
                t = xh0 if dk < 8 else xh1
                return t[:, dk % 8, :]

            # attention thunks of the previous step, spread over this step's
            # projection chains
            th = attn_thunks(idx - 1) if idx > 0 else []
            tpos = 0

            def run_thunks(target):
                nonlocal tpos
                while tpos < min(target, len(th)):
                    th[tpos]()
                    tpos += 1

            # ---- Q/K transposed projections + RMSNorm + RoPE ----
            # PE pipelining: after each projection chain, emit the previous
            # chain's sum-of-squares matmul and the chain before that's
            # rotate matmul, so PE never waits on ACT/DVE.
            qr_tiles = {}
            qrs[idx] = qr_tiles
            states = []

            def emit_ss(stt):
                ssps = psS.tile([128, SG], F32, tag="score")
                nc.tensor.matmul(ssps[:], ones_t[:], stt["qsq"][:],
                                 start=True, stop=True)
                # 1/sqrt(v) = exp(-0.5*ln(v)) keeps every ACT func in the
                # natural_log_exp_and_others table set (no table thrash).
                rstd = rpool.tile([128, SG], F32, tag="rstd")
                nc.scalar.activation(rstd[:], ssps[:], AF.Ln,
                                     bias=eps_t[:, 0:1], scale=1.0 / HD)
                nc.scalar.activation(rstd[:], rstd[:], AF.Exp, scale=-0.5)
                qn = rpool.tile([128, SG], BF16, tag="qn")
                nc.vector.tensor_mul(qn[:], stt["ps"][:], rstd[:])
                t1 = rpool.tile([128, SG], BF16, tag="t1")
                cost = cqt if stt["t"] == "q" else ckt
                nc.vector.tensor_mul(t1[:], qn[:], cost[:])
                stt["qn"] = qn
                stt["t1"] = t1

            def emit_rot(stt):
                # rotate_half on GpSimd (SW cores may read a different
                # partition base than they write, as long as both INPUTS
                # share a base): dst[p] = qn[(p+64)%128] * sin_signed[p].
                # The sin tables are partition-rolled by 64 and sign-folded
                # on the host so both inputs align at the same base.
                sint = sqt if stt["t"] == "q" else skt
                dst = stt["dst"]
                qn = stt["qn"]
                nc.gpsimd.tensor_mul(dst[0:64, :], qn[64:128, :],
                                     sint[64:128, :])
                nc.gpsimd.tensor_mul(dst[64:128, :], qn[0:64, :],
                                     sint[0:64, :])
                nc.vector.tensor_add(dst[:], dst[:], stt["t1"][:])

            chains = [("q", 0), ("k", 0), ("q", 1), ("k", 1),
                      ("v", 0), ("v", 1), ("v", 2), ("v", 3)]
            for i, (t, h) in enumerate(chains):
                if t in ("q", "k"):
                    w_t = wq_t if t == "q" else wk_t
                    ps = psA.tile([128, SG], F32, tag="a")
                    for dk in range(NDK):
                        nc.tensor.matmul(
                            ps[:], w_t[:, dk, h * HD:(h + 1) * HD], xs(dk),
                            start=(dk == 0), stop=(dk == NDK - 1))
                    qsq = rpool.tile([128, SG], BF16, tag="qsq")
                    nc.scalar.activation(qsq[:], ps[:], AF.Square)
                    if t == "q":
                        dst = qpool.tile([128, SG], BF16, tag="qr")
                        qr_tiles[h] = dst
                    else:
                        dst = kpool.tile([128, SG], BF16, tag="kr")
                        KrT[(b, h, g)] = dst
                    states.append({"ps": ps, "qsq": qsq, "t": t, "dst": dst})
                else:
                    st = h
                    psv = psA.tile([128, NL], F32, tag="a")
                    for dk in range(NDK):
                        nc.tensor.matmul(
                            psv[:], xs(dk)[:, st * 128:(st + 1) * 128],
                            wv_t[:, dk, :],
                            start=(dk == 0), stop=(dk == NDK - 1))
                    vt = vpool.tile([128, NL], BF16, tag="v")
                    nc.vector.tensor_copy(vt[:], psv[:])
                    Vt[(b, 4 * g + st)] = vt
                if 0 <= i - 1 < 4:
                    emit_ss(states[i - 1])
                if 0 <= i - 2 < 4:
                    emit_rot(states[i - 2])
                run_thunks((len(th) * (i + 1)) // len(chains))

            run_thunks(len(th))

            # Wo of step idx-2 (its attention completed during step idx-1)
            if idx - 2 >= 0:
                for t in wo_thunks(idx - 2):
                    t()

        # drain: attention of the last step, then the last two Wo blocks
        for t in attn_thunks(NSTEP - 1) + wo_thunks(NSTEP - 2):
            t()
        for t in wo_thunks(NSTEP - 1):
            t()

    nc.compile()
    return nc


def _host_prep(hidden_states, cos, sin, Wq, Wk, Wv, Wo, q_scale, k_scale):
    f32 = np.float32
    bf16 = ml_dtypes.bfloat16
    hs = np.asarray(hidden_states, f32)
    cos = np.asarray(cos, f32)
    sin = np.asarray(sin, f32)
    qs = np.asarray(q_scale, f32)
    ks = np.asarray(k_scale, f32)

    def ptile(a2d, width):
        """[128*K, W] -> [128, K*W] partition-major contiguous pre-tiling."""
        k = a2d.shape[0] // 128
        return np.ascontiguousarray(
            a2d.reshape(k, 128, width).transpose(1, 0, 2).reshape(128, -1)
        ).astype(bf16)

    # xt: per (b,g) block of X^T, pre-tiled
    xt = np.stack([
        ptile(hs[b].T[:, g * SG:(g + 1) * SG], SG)
        for b in range(B) for g in range(G)
    ])   # [B*G, 128, 16*SG]

    qs_rot = np.roll(qs, -64)
    ks_rot = np.roll(ks, -64)
    # rotate_half's -1 on the first half is folded into the sin tables,
    # which are then partition-rolled by 64 so the GpSimd rotate muls read
    # both inputs (qn, sin) at the same partition base.
    sgn = np.ones((HD, 1), f32)
    sgn[:64] = -1.0
    cq_full = [(cos[b] * qs[None, :] * SCALE).T for b in range(B)]    # [HD,S]
    sq_full = [np.roll((sin[b] * qs_rot[None, :] * SCALE).T * sgn, 64, axis=0)
               for b in range(B)]
    ck_full = [(cos[b] * ks[None, :]).T for b in range(B)]
    sk_full = [np.roll((sin[b] * ks_rot[None, :]).T * sgn, 64, axis=0)
               for b in range(B)]
    cs_all = np.stack([
        np.concatenate([t[:, g * SG:(g + 1) * SG]
                        for t in (cq_full[b], sq_full[b],
                                  ck_full[b], sk_full[b])], axis=1)
        for b in range(B) for g in range(G)
    ]).astype(bf16)   # [B*G, 128, 4*SG]
    cs_all = np.ascontiguousarray(cs_all)

    ones = np.ones((128, 128), bf16)
    kk = np.arange(128)[:, None]
    qq = np.arange(128)[None, :]
    tri = np.concatenate([(qq >= kk).astype(f32),     # causal keep
                          (kk >= qq).astype(f32)],    # window-edge keep
                         axis=1).astype(bf16)
    shared = {"xt": xt, "cs": cs_all, "ones_d": ones, "tri_d": tri}
    Wq = np.asarray(Wq, f32)
    Wk = np.asarray(Wk, f32)
    Wv = np.asarray(Wv, f32)
    Wo = np.asarray(Wo, f32)
    in_maps = []
    for c in range(NCORES):
        m = dict(shared)
        m["wq"] = ptile(Wq[:, c * NL:(c + 1) * NL], NL)
        m["wk"] = ptile(Wk[:, c * NL:(c + 1) * NL], NL)
        m["wv"] = ptile(Wv[:, c * NL:(c + 1) * NL], NL)
        m["wo"] = ptile(Wo[c * NL:(c + 1) * NL, :], D)
        in_maps.append(m)
    return in_maps


def get_nc():
    if "nc" not in _CACHE:
        _CACHE["nc"] = _build()
    return _CACHE["nc"]


def kernel(hidden_states, cos, sin, Wq, Wk, Wv, Wo, q_scale, k_scale):
    nc = get_nc()
    in_maps = _host_prep(hidden_states, cos, sin, Wq, Wk, Wv, Wo,
                         q_scale, k_scale)
    res = bass_utils.run_bass_kernel_spmd(nc, in_maps,
                                          core_ids=list(range(NCORES)))
    acc = np.zeros((B * S, D), np.float64)
    for r in res.results:
        acc += r["opart"].astype(np.float64)
    return np.ascontiguousarray(
        acc.reshape(B, S, D).astype(np.float32))


# revision 63
# speedup vs baseline: 1.1393x; 1.0232x over previous
"""Trainium2 Bass kernel for nn_MultiHeadAttention_8400956031164.

Full attention block: QKV proj + per-head RMSNorm + RoPE + sliding-window
causal attention (WIN=1024) + output proj.

Sharding: tensor-parallel over heads across 8 cores (2 heads/core), both
batches looped per core. Host sums the 8 partial Wo outputs.

Device-side layout strategy (per core):
  - X^T [D, S] streamed per 512-token group; Q,K produced TRANSPOSED
    [hd=128, s] per head directly from projection (lhsT = W slice).
  - All matmul operands in bf16 (PSUM accumulation f32): measured end-to-end
    rounding impact ~3.8e-3 rel err vs the 2e-2 gate; bf16 runs 1 cycle/row
    at any output width (f32r degrades 4x under 256) and halves DMA + SBUF +
    LDWEIGHTS time.
  - RMSNorm in transposed layout: sum(q^2) over hd via all-ones matmul
    (broadcast across partitions in PSUM), 1/sqrt via exp(-0.5*ln) on ACT.
  - RoPE in transposed layout: rotate_half runs on GpSimd (its SW cores
    may read a shifted partition base), with the sign AND the 64-row
    partition roll folded into the host sin tables; keeps the PE free.
  - Scores computed transposed S^T[k, q] (k on partitions) per 128x(<=512)
    block over the sliding window; exp on ACT (PSUM->SBUF, bf16 out);
    causal/window triangle masks applied as 0/1-table multiplies on DVE
    (GpSimd's in-order queue is busy with the rotates).
  - Softmax denominator via all-ones matmul accumulation; 1/L via the
    single-pass DVE reciprocal_approx_fast; PV accumulates V^T @ P^T =
    out^T [hd, q] in PSUM with variable-N has_written semantics.
  - Wo: lhsT = normalized out^T slices, accumulate 2 head-chunks, ACT/DVE
    copy PSUM->SBUF (bf16), DMA out bf16; host sums partials in f64.

Software pipeline (the key to keeping PE ~90% busy): per 512-token step N
the emission order is [input-DMA prefetch for N+2] + [proj chains of N,
with the attention blocks of step N-1 interleaved between chains] + [Wo
of step N-2]. The ACT exp stream of step N-1 then overlaps the PE
projection matmuls of step N instead of serializing behind them, and the
PE never waits on the softmax-normalization (DVE) tail. Deep tile pools
(x 6, out 6) decouple both DMA directions from compute: input transfers
start two steps before use, and Wo's PSUM->SBUF copies never wait on the
output-DMA drain. PSUM banks: 3 proj/Wo + 2 scores + 2 out + 1 denom.
"""

import functools

import numpy as np
import ml_dtypes
from contextlib import ExitStack

import concourse.bass as bass
import concourse.tile as tile
import concourse.mybir as mybir
import concourse.hw_specs as _hw_specs
from concourse import bacc, bass_utils


def _patch_act_tables():
    """Steer every activation in this kernel (Square/Ln/Exp/Copy) to the one
    ACT table set that really contains them all (natural_log_exp_and_others),
    so the greedy first-containing-set chooser never thrashes table loads.
    Only *removes* candidate sets; chosen ids still match act_info.json."""
    if getattr(_patch_act_tables, "done", False):
        return
    orig = _hw_specs.get_activation_tables
    AFT = mybir.ActivationFunctionType
    drop = {AFT.Exp, AFT.Ln, AFT.Square, AFT.Copy, AFT.Identity}

    @functools.cache
    def patched(module_arch):
        t = {k: set(v) for k, v in orig(module_arch).items()}
        for name, funcs in t.items():
            if name != "natural_log_exp_and_others":
                funcs -= drop
        return t

    _hw_specs.get_activation_tables = patched
    bacc.get_activation_tables = patched
    _patch_act_tables.done = True


_patch_act_tables()

B, S, D, H, HD, WIN = 2, 2048, 2048, 16, 128, 1024
EPS = 1e-6
SCALE = HD ** -0.5
NCORES = 8
HLOC = H // NCORES          # heads per core = 2
NL = HLOC * HD              # local head dims = 256
SG = 512                    # token group size
G = S // SG                 # groups per batch = 4
NDK = D // 128              # contraction chunks = 16

F32 = mybir.dt.float32
BF16 = mybir.dt.bfloat16
AF = mybir.ActivationFunctionType

_CACHE = {}


def _build():
    nc = bacc.Bacc(trn_type="TRN2", target_bir_lowering=False, debug=False)

    def din(name, shape, dt):
        return nc.dram_tensor(name, shape, dt, kind="ExternalInput").ap()

    # All inputs are host-pre-tiled to be partition-major contiguous so every
    # DMA is ~128 descriptors of large contiguous runs.
    xt = din("xt", [B * G, 128, NDK * SG], BF16)      # per (b,g) [128, 16*512]
    wq = din("wq", [128, NDK * NL], BF16)
    wk = din("wk", [128, NDK * NL], BF16)
    wv = din("wv", [128, NDK * NL], BF16)
    wo = din("wo", [128, HLOC * D], BF16)
    cs = din("cs", [B * G, 128, 4 * SG], BF16)        # packed cq|sq|ck|sk
    ones_d = din("ones_d", [128, 128], BF16)
    tri_d = din("tri_d", [128, 256], BF16)   # causal-keep | window-keep 0/1
    opart = nc.dram_tensor("opart", [B * S, D], BF16, kind="ExternalOutput").ap()

    steps = [(b, g) for b in range(B) for g in range(G)]
    NSTEP = len(steps)

    with tile.TileContext(nc) as tc, ExitStack() as ctx:
        const = ctx.enter_context(tc.tile_pool(name="const", bufs=1))
        wpool = ctx.enter_context(tc.tile_pool(name="w", bufs=1))
        xpool = ctx.enter_context(tc.tile_pool(name="x", bufs=6))
        cspool = ctx.enter_context(tc.tile_pool(name="cs", bufs=3))
        qpool = ctx.enter_context(tc.tile_pool(name="qr", bufs=5))
        kpool = ctx.enter_context(tc.tile_pool(name="kr", bufs=10))
        vpool = ctx.enter_context(tc.tile_pool(name="v", bufs=18))
        rpool = ctx.enter_context(tc.tile_pool(name="rms", bufs=2))
        ppool = ctx.enter_context(tc.tile_pool(name="p", bufs=5))
        lpool = ctx.enter_context(tc.tile_pool(name="lin", bufs=2))
        opool = ctx.enter_context(tc.tile_pool(name="osb", bufs=6))
        outp = ctx.enter_context(tc.tile_pool(name="out", bufs=6))
        psA = ctx.enter_context(tc.tile_pool(name="psA", bufs=3, space="PSUM"))
        psS = ctx.enter_context(tc.tile_pool(name="psS", bufs=2, space="PSUM"))
        psO = ctx.enter_context(tc.tile_pool(name="psO", bufs=2, space="PSUM"))
        psL = ctx.enter_context(tc.tile_pool(name="psL", bufs=1, space="PSUM"))

        inputs = {}    # idx -> (xh0, xh1, cst)
        qrs = {}       # idx -> {h: [128,SG] bf16}
        KrT = {}       # (b, h, g) -> [128,SG] bf16
        Vt = {}        # (b, st_abs) -> [128,NL] bf16
        osbs_all = {}  # idx -> {h: [128,SG] bf16}

        def issue_dmas(idx, spread=False):
            """Input DMAs for one step. `spread` (startup only) issues the
            chunks from different engines for parallel DMA queues."""
            b, g = steps[idx]
            bg = b * G + g
            src = xt[bg].rearrange("p (a s) -> p a s", s=SG)
            xh0 = xpool.tile([128, 8, SG], BF16, tag="xt", name="xh0")
            xh1 = xpool.tile([128, 8, SG], BF16, tag="xt", name="xh1")
            if spread:
                nc.sync.dma_start(xh0[:, 0:3, :], src[:, 0:3, :])
                nc.gpsimd.dma_start(xh0[:, 3:5, :], src[:, 3:5, :])
                nc.scalar.dma_start(xh0[:, 5:8, :], src[:, 5:8, :])
                nc.gpsimd.dma_start(xh1[:], src[:, 8:16, :])
            else:
                nc.sync.dma_start(xh0[:], src[:, 0:8, :])
                nc.sync.dma_start(xh1[:], src[:, 8:16, :])
            cst = cspool.tile([128, 4, SG], BF16, tag="cs")
            nc.sync.dma_start(
                cst[:], cs[bg].rearrange("p (f s) -> p f s", s=SG))
            inputs[idx] = (xh0, xh1, cst)

        def attn_thunks(idx):
            """Attention micro-ops (PE-centric) for step idx, as a list of
            thunks to interleave between the next step's projection chains."""
            b, g = steps[idx]
            qr_tiles = qrs[idx]
            thunks = []
            osbs = {}
            osbs_all[idx] = osbs

            def make_head(h):
                qr_t = qr_tiles[h]
                kts = list(range(max(0, 4 * g - 8), 4 * g + 4))
                nk = len(kts)
                state = {}
                pend = []
                LAG = 3

                def start():
                    state["oacc"] = psO.tile([128, SG], F32, tag="o",
                                             name="oacc")
                    state["lacc"] = psL.tile([128, SG], F32, tag="l",
                                             name="lacc")

                def emit_pv(item, first, last):
                    kt, qoff, n, p = item
                    nc.tensor.matmul(
                        state["oacc"][:, qoff:qoff + n],
                        Vt[(b, kt)][:, h * HD:(h + 1) * HD], p[:],
                        start=first, stop=last)
                    nc.tensor.matmul(
                        state["lacc"][:, qoff:qoff + n], ones_t[:], p[:],
                        start=first, stop=last)

                def block(i):
                    kt = kts[i]
                    qt_lo = max(4 * g, kt)
                    qt_hi = min(4 * g + 3, kt + 8)
                    qoff = 128 * (qt_lo - 4 * g)
                    n = 128 * (qt_hi - qt_lo + 1)
                    sc = psS.tile([128, n], F32, tag="score")
                    kr_t = KrT[(b, h, kt // 4)]
                    c = (kt % 4) * 128
                    nc.tensor.matmul(sc[:], kr_t[:, c:c + 128],
                                     qr_t[:, qoff:qoff + n],
                                     start=True, stop=True)
                    p = ppool.tile([128, n], BF16, tag="p")
                    nc.scalar.activation(p[:], sc[:], AF.Exp)
                    if kt >= 4 * g:
                        # causal triangle: keep kk <= qq (0/1 mul on DVE —
                        # GpSimd is busy with the rotate muls)
                        nc.vector.tensor_mul(p[:, 0:128], p[:, 0:128],
                                             tri_t[:, 0:128])
                    if kt + 8 <= 4 * g + 3:
                        # window edge: keep kk >= qq
                        nc.vector.tensor_mul(p[:, n - 128:n],
                                             p[:, n - 128:n],
                                             tri_t[:, 128:256])
                    pend.append((kt, qoff, n, p))
                    if i >= LAG:
                        emit_pv(pend[i - LAG], first=(i - LAG == 0), last=False)

                def tail():
                    for j in range(max(0, nk - LAG), nk):
                        emit_pv(pend[j], first=(j == 0), last=(j == nk - 1))
                    linv = lpool.tile([128, SG], F32, tag="lin")
                    nc.vector.reciprocal_approx_fast(linv[:], state["lacc"][:])
                    osb = opool.tile([128, SG], BF16, tag="osb")
                    nc.vector.tensor_mul(osb[:], state["oacc"][:], linv[:])
                    osbs[h] = osb

                thunks.append(start)
                for i in range(nk):
                    thunks.append(lambda i=i: block(i))
                thunks.append(tail)

            for h in range(HLOC):
                make_head(h)
            return thunks

        def wo_thunks(idx):
            """Wo micro-ops for step idx as thunks: one per (st, dg) psum
            pair, plus the output DMA after each st row completes."""
            b, g = steps[idx]
            s0 = b * S + g * SG
            osbs_ = osbs_all.pop(idx)
            thunks = []
            state = {}

            def pair(st, dg):
                if dg == 0:
                    state["ot"] = outp.tile([128, D], BF16, tag="out",
                                            name="ot")
                pso = psA.tile([128, 512], F32, tag="a")
                nc.tensor.matmul(pso[:],
                                 osbs_[0][:, st * 128:(st + 1) * 128],
                                 wo_t[:, 0, dg * 512:(dg + 1) * 512],
                                 start=True, stop=False)
                nc.tensor.matmul(pso[:],
                                 osbs_[1][:, st * 128:(st + 1) * 128],
                                 wo_t[:, 1, dg * 512:(dg + 1) * 512],
                                 start=False, stop=True)
                dst = state["ot"][:, dg * 512:(dg + 1) * 512]
                if dg % 2 == 0:
                    nc.scalar.copy(dst, pso[:])
                else:
                    nc.vector.tensor_copy(dst, pso[:])
                if dg == 3:
                    row = s0 + st * 128
                    nc.sync.dma_start(opart[row:row + 128, :],
                                      state["ot"][:])

            for st in range(4):
                for dg in range(4):
                    thunks.append(lambda st=st, dg=dg: pair(st, dg))
            return thunks

        # DMA issue order = first-use order, split into small tiles so the
        # PE's first projection matmuls start as soon as the first x / wq
        # chunks land (deps are tile-granular). Startup chunks ride
        # different engines' issue queues for parallel DMA; wo_t is not
        # needed until step 2.
        wqsrc = wq.rearrange("p (a n) -> p a n", n=NL)
        wq_t = wpool.tile([128, NDK, NL], BF16, tag="wq")
        nc.sync.dma_start(wq_t[:, 0:6, :], wqsrc[:, 0:6, :])
        nc.gpsimd.dma_start(wq_t[:, 6:11, :], wqsrc[:, 6:11, :])
        nc.scalar.dma_start(wq_t[:, 11:16, :], wqsrc[:, 11:16, :])
        issue_dmas(0, spread=True)
        wk_t = wpool.tile([128, NDK, NL], BF16, tag="wk")
        nc.sync.dma_start(wk_t[:], wk.rearrange("p (a n) -> p a n", n=NL))
        ones_t = const.tile([128, 128], BF16, tag="ones")
        nc.sync.dma_start(ones_t[:], ones_d)
        eps_t = const.tile([128, 1], F32, tag="eps")
        nc.vector.memset(eps_t[:], EPS)
        wv_t = wpool.tile([128, NDK, NL], BF16, tag="wv")
        nc.sync.dma_start(wv_t[:], wv.rearrange("p (a n) -> p a n", n=NL))
        tri_t = const.tile([128, 256], BF16, tag="tri")
        nc.sync.dma_start(tri_t[:], tri_d)
        issue_dmas(1)
        wo_t = wpool.tile([128, HLOC, D], BF16, tag="wo")
        nc.sync.dma_start(wo_t[:], wo.rearrange("p (c d) -> p c d", d=D))
        issue_dmas(2)
        for idx in range(NSTEP):
            b, g = steps[idx]
            if 1 <= idx and idx + 2 < NSTEP:
                issue_dmas(idx + 2)
            xh0, xh1, cst = inputs.pop(idx)
            cqt = cst[:, 0, :]
            sqt = cst[:, 1, :]
            ckt = cst[:, 2, :]
            skt = cst[:, 3, :]

            def xs(dk):
                t = xh0 if dk < 8 else xh1
                return t[:, dk % 8, :]

            # attention thunks of the previous step, spread over this step's
            # projection chains
            th = attn_thunks(idx - 1) if idx > 0 else []
            tpos = 0

            def run_thunks(target):
                nonlocal tpos
                while tpos < min(target, len(th)):
                    th[tpos]()
                    tpos += 1

            # ---- Q/K transposed projections + RMSNorm + RoPE ----
            # PE pipelining: after each projection chain, emit the previous
            # chain's sum-of-squares matmul and the chain before that's
            # rotate matmul, so PE never waits on ACT/DVE.
            qr_tiles = {}
            qrs[idx] = qr_tiles
            states = []

            def emit_ss(stt):
                ssps = psS.tile([128, SG], F32, tag="score")
                nc.tensor.matmul(ssps[:], ones_t[:], stt["qsq"][:],
                                 start=True, stop=True)
                # 1/sqrt(v) = exp(-0.5*ln(v)) keeps every ACT func in the
                # natural_log_exp_and_others table set (no table thrash).
                rstd = rpool.tile([128, SG], F32, tag="rstd")
                nc.scalar.activation(rstd[:], ssps[:], AF.Ln,
                                     bias=eps_t[:, 0:1], scale=1.0 / HD)
                nc.scalar.activation(rstd[:], rstd[:], AF.Exp, scale=-0.5)
                qn = rpool.tile([128, SG], BF16, tag="qn")
                nc.vector.tensor_mul(qn[:], stt["ps"][:], rstd[:])
                t1 = rpool.tile([128, SG], BF16, tag="t1")
                cost = cqt if stt["t"] == "q" else ckt
                nc.vector.tensor_mul(t1[:], qn[:], cost[:])
                stt["qn"] = qn
                stt["t1"] = t1

            def emit_rot(stt):
                # rotate_half on GpSimd (SW cores may read a different
                # partition base than they write, as long as both INPUTS
                # share a base): dst[p] = qn[(p+64)%128] * sin_signed[p].
                # The sin tables are partition-rolled by 64 and sign-folded
                # on the host so both inputs align at the same base.
                sint = sqt if stt["t"] == "q" else skt
                dst = stt["dst"]
                qn = stt["qn"]
                nc.gpsimd.tensor_mul(dst[0:64, :], qn[64:128, :],
                                     sint[64:128, :])
                nc.gpsimd.tensor_mul(dst[64:128, :], qn[0:64, :],
                                     sint[0:64, :])
                nc.vector.tensor_add(dst[:], dst[:], stt["t1"][:])

            chains = [("q", 0), ("k", 0), ("q", 1), ("k", 1),
                      ("v", 0), ("v", 1), ("v", 2), ("v", 3)]
            for i, (t, h) in enumerate(chains):
                if t in ("q", "k"):
                    w_t = wq_t if t == "q" else wk_t
                    ps = psA.tile([128, SG], F32, tag="a")
                    for dk in range(NDK):
                        nc.tensor.matmul(
                            ps[:], w_t[:, dk, h * HD:(h + 1) * HD], xs(dk),
                            start=(dk == 0), stop=(dk == NDK - 1))
                    qsq = rpool.tile([128, SG], BF16, tag="qsq")
                    nc.scalar.activation(qsq[:], ps[:], AF.Square)
                    if t == "q":
                        dst = qpool.tile([128, SG], BF16, tag="qr")
                        qr_tiles[h] = dst
                    else:
                        dst = kpool.tile([128, SG], BF16, tag="kr")
                        KrT[(b, h, g)] = dst
                    states.append({"ps": ps, "qsq": qsq, "t": t, "dst": dst})
                else:
                    st = h
                    psv = psA.tile([128, NL], F32, tag="a")
                    for dk in range(NDK):
                        nc.tensor.matmul(
                            psv[:], xs(dk)[:, st * 128:(st + 1) * 128],
                            wv_t[:, dk, :],
                            start=(dk == 0), stop=(dk == NDK - 1))
                    vt = vpool.tile([128, NL], BF16, tag="v")
                    nc.vector.tensor_copy(vt[:], psv[:])
                    Vt[(b, 4 * g + st)] = vt
                if 0 <= i - 1 < 4:
                    emit_ss(states[i - 1])
                if 0 <= i - 2 < 4:
                    emit_rot(states[i - 2])
                run_thunks((len(th) * (i + 1)) // len(chains))

            run_thunks(len(th))

            # Wo of step idx-2 (its attention completed during step idx-1)
            if idx - 2 >= 0:
                for t in wo_thunks(idx - 2):
                    t()

        # drain: attention of the last step, then the last two Wo blocks
        for t in attn_thunks(NSTEP - 1) + wo_thunks(NSTEP - 2):
            t()
        for t in wo_thunks(NSTEP - 1):
            t()

    nc.compile()
    return nc


def _host_prep(hidden_states, cos, sin, Wq, Wk, Wv, Wo, q_scale, k_scale):
    f32 = np.float32
    bf16 = ml_dtypes.bfloat16
    hs = np.asarray(hidden_states, f32)
    cos = np.asarray(cos, f32)
    sin = np.asarray(sin, f32)
    qs = np.asarray(q_scale, f32)
    ks = np.asarray(k_scale, f32)

    def ptile(a2d, width):
        """[128*K, W] -> [128, K*W] partition-major contiguous pre-tiling."""
        k = a2d.shape[0] // 128
        return np.ascontiguousarray(
            a2d.reshape(k, 128, width).transpose(1, 0, 2).reshape(128, -1)
        ).astype(bf16)

    # xt: per (b,g) block of X^T, pre-tiled
    xt = np.stack([
        ptile(hs[b].T[:, g * SG:(g + 1) * SG], SG)
        for b in range(B) for g in range(G)
    ])   # [B*G, 128, 16*SG]

    qs_rot = np.roll(qs, -64)
    ks_rot = np.roll(ks, -64)
    # rotate_half's -1 on the first half is folded into the sin tables,
    # which are then partition-rolled by 64 so the GpSimd rotate muls read
    # both inputs (qn, sin) at the same partition base.
    sgn = np.ones((HD, 1), f32)
    sgn[:64] = -1.0
    cq_full = [(cos[b] * qs[None, :] * SCALE).T for b in range(B)]    # [HD,S]
    sq_full = [np.roll((sin[b] * qs_rot[None, :] * SCALE).T * sgn, 64, axis=0)
               for b in range(B)]
    ck_full = [(cos[b] * ks[None, :]).T for b in range(B)]
    sk_full = [np.roll((sin[b] * ks_rot[None, :]).T * sgn, 64, axis=0)
               for b in range(B)]
    cs_all = np.stack([
        np.concatenate([t[:, g * SG:(g + 1) * SG]
                        for t in (cq_full[b], sq_full[b],
                                  ck_full[b], sk_full[b])], axis=1)
        for b in range(B) for g in range(G)
    ]).astype(bf16)   # [B*G, 128, 4*SG]
    cs_all = np.ascontiguousarray(cs_all)

    ones = np.ones((128, 128), bf16)
    kk = np.arange(128)[:, None]
    qq = np.arange(128)[None, :]
    tri = np.concatenate([(qq >= kk).astype(f32),     # causal keep
                          (kk >= qq).astype(f32)],    # window-edge keep
                         axis=1).astype(bf16)
    shared = {"xt": xt, "cs": cs_all, "ones_d": ones, "tri_d": tri}
    Wq = np.asarray(Wq, f32)
    Wk = np.asarray(Wk, f32)
    Wv = np.asarray(Wv, f32)
    Wo = np.asarray(Wo, f32)
    in_maps = []
    for c in range(NCORES):
        m = dict(shared)
        m["wq"] = ptile(Wq[:, c * NL:(c + 1) * NL], NL)
        m["wk"] = ptile(Wk[:, c * NL:(c + 1) * NL], NL)
        m["wv"] = ptile(Wv[:, c * NL:(c + 1) * NL], NL)
        m["wo"] = ptile(Wo[c * NL:(c + 1) * NL, :], D)
        in_maps.append(m)
    return in_maps


def get_nc():
    if "nc" not in _CACHE:
        _CACHE["nc"] = _build()
    return _CACHE["nc"]


def kernel(hidden_states, cos, sin, Wq, Wk, Wv, Wo, q_scale, k_scale):
    nc = get_nc()
    in_maps = _host_prep(hidden_states, cos, sin, Wq, Wk, Wv, Wo,
                         q_scale, k_scale)
    res = bass_utils.run_bass_kernel_spmd(nc, in_maps,
                                          core_ids=list(range(NCORES)))
    acc = np.zeros((B * S, D), np.float64)
    for r in res.results:
        acc += r["opart"].astype(np.float64)
    return np.ascontiguousarray(
        acc.reshape(B, S, D).astype(np.float32))
